# revision 1
# baseline (speedup 1.0000x reference)
"""Boundary-loss Trainium2 kernel (Bass/Tile), SPMD over 8 NeuronCores.

Problem: loss = mean(softmax(logits, C) * phi(targets)) with phi the signed
Euclidean distance map of each class mask:
    phi_c = sqrt(edt2(mask_c)) - sqrt(edt2(~mask_c)) + 1   (non-degenerate case)

Key algorithmic facts used:
  * the C=4 class masks partition the image (one-hot targets), so
    edt2(~mask_c) = min_{c' != c} edt2(mask_{c'}) -- only 4 EDTs per image.
  * per pixel p with target class t: R_t(p) = 0, hence
      sum_c probs_c * phi_c = (sum_c e_c R_c + S_e - e_t * m2) / S_e
    with e_c = exp(logit_c), S_e = sum e_c, R_c = sqrt(edt2(mask_c)),
    m2 = second-smallest R at p, e_t = e of the target class.
  * exact EDT on-device:
      row pass: 1-D L1 distance via two tensor_tensor_scans (fw + reversed bw),
        squared -> d1 (bf16: every value that can win a min is exact).
      col pass: windowed parabolic min-plus cur = min(cur, d1[h+-d] + d^2),
        exact iff window K >= max Euclidean distance. The ACT engine computes
        d1 + d^2 (bias-add), the DVE does bf16 2x-mode mins on 4B-aligned
        slices (odd shifts read an ACT tile built from d1[j+1]). The kernel
        returns max(D) per core; the host verifies max(D) <= K^2
        (certificate) and recompiles with a bigger K (or falls back to an
        exact host path) on violation -- for the 25%-density random masks of
        this problem the max distance is 5, so K=6 has margin.

Engine split: DVE scans/mins/compares, ACT shifted adds + exp + sqrt-on-PSUM-
eviction, PE 128x128 transposes, GPSIMD the add/mult combine chains.

Sharding: data-parallel over B=8, one batch item per core. Each core returns
128 per-partition partial sums + the max-D certificate; the host does the
final (tiny) reduction.
"""
from contextlib import ExitStack

import numpy as np

import concourse.bass as bass
import concourse.tile as tile
from concourse import bacc, mybir
from concourse.bass_utils import run_bass_kernel_spmd
from concourse.masks import make_identity

P = 128          # SBUF partitions
C = 4            # classes
H = W = 384
KCH = H // P     # 3 row-chunks
N_CORES = 8
BIG = 65536.0    # 1-D distance sentinel (exact in bf16; squared ~4.3e9)
DEFAULT_K = 5    # parabolic window; certified at runtime

FP32 = mybir.dt.float32
BF16 = mybir.dt.bfloat16
INT32 = mybir.dt.int32
OP = mybir.AluOpType
ACT = mybir.ActivationFunctionType


def _build_nc(K: int) -> bass.Bass:
    nc = bacc.Bacc("TRN2", target_bir_lowering=False, debug=False)
    logits_d = nc.dram_tensor("logits", [C, H, W], FP32, kind="ExternalInput")
    targets_d = nc.dram_tensor("targets", [H, W], INT32, kind="ExternalInput")
    out_d = nc.dram_tensor("out", [P, 2], FP32, kind="ExternalOutput")

    with tile.TileContext(nc) as tc, ExitStack() as ctx:
        pool = ctx.enter_context(tc.tile_pool(name="main", bufs=1))
        psum_pool = ctx.enter_context(tc.tile_pool(name="ps", bufs=2, space="PSUM"))
        psum_pool_b = ctx.enter_context(
            tc.tile_pool(name="psb", bufs=2, space="PSUM"))

        # ---- loads ----
        T = pool.tile([P, KCH, W], INT32)
        tr = targets_d[:].rearrange("(k p) w -> p k w", p=P)
        for k in range(KCH):
            nc.sync.dma_start(T[:, k], tr[:, k])
        L = pool.tile([P, C, KCH, W], FP32)
        nc.sync.dma_start(L[:], logits_d[:].rearrange("c (k p) w -> p c k w", p=P))

        # ---- constants ----
        ONES = pool.tile([P, W], BF16)
        nc.vector.memset(ONES[:], 1.0)
        IDENT = pool.tile([P, P], BF16)
        make_identity(nc, IDENT[:])
        # per-delta squared-shift bias columns for the ACT adds
        BIASQ = pool.tile([P, K], FP32)
        for d in range(1, K + 1):
            nc.vector.memset(BIASQ[:, d - 1:d], float(d * d))

        # ---- masks: F_c = (t != c) * BIG  (0 on class-c pixels) ----
        TF = pool.tile([P, KCH, W], FP32)
        for k in range(KCH):
            nc.scalar.copy(TF[:, k], T[:, k])  # int32 -> fp32
        F = pool.tile([P, C, KCH, W], BF16)
        for k in range(KCH):
            nc.vector.tensor_scalar(F[:, 0, k], TF[:, k], 0.0, BIG,
                                    op0=OP.not_equal, op1=OP.mult)
        for c in range(1, C):
            nc.vector.tensor_scalar(F[:, c], TF[:], float(c), BIG,
                                    op0=OP.not_equal, op1=OP.mult)

        # ---- row pass: 1-D L1 distance along W (fw+bw scans), squared ----
        FW = pool.tile([P, C, KCH, W], BF16)
        BW = pool.tile([P, C, KCH, W], BF16)
        D1T = BW  # aliased below after the scans' BW use ends
        CUR = pool.tile([P, C, KCH, W], BF16)
        TMPB = pool.tile([P, C, KCH, W], BF16)
        for c in range(C):
            for k in range(KCH):
                nc.vector.tensor_tensor_scan(
                    FW[:, c, k, :], ONES[:], F[:, c, k, :], BIG,
                    op0=OP.add, op1=OP.min)
                nc.vector.tensor_tensor_scan(
                    BW[:, c, k, ::-1], ONES[:], F[:, c, k, ::-1], BIG,
                    op0=OP.add, op1=OP.min)
            nc.vector.tensor_tensor(FW[:, c], FW[:, c], BW[:, c],
                                    op=OP.min)                    # rho_c

        # ---- transposes per class (PE/ACT run while DVE still scans the
        # later classes), each followed by that class's CUR init + delta=1
        # mins queued on DVE after all scan work ----
        for c in range(C):
            ps9 = psum_pool.tile([P, KCH, KCH, P], BF16, tag="pst")
            for kh in range(KCH):
                for kw in range(KCH):
                    nc.tensor.matmul(ps9[:, kw, kh, :],
                                     FW[:, c, kh, kw * P:(kw + 1) * P],
                                     IDENT[:], is_transpose=True)
            nc.scalar.activation(
                D1T[:, c],
                ps9[:].rearrange("p kw kh x -> p kw (kh x)"),
                ACT.Square)   # d1 = rho^2 rides the eviction
            nc.scalar.activation(TMPB[:, c, :, 0:W - 1], D1T[:, c, :, 1:W],
                                 ACT.Identity, bias=BIASQ[:, 0:1], scale=1.0)
        # ---- e_c = exp(logit_c) (early: feeds the gpsimd e_t chain) ----
        E = pool.tile([P, C, KCH, W], FP32)
        for c in range(C):
            nc.scalar.activation(E[:, c], L[:, c], ACT.Exp)

        # ind_c = [t == c] from the masks (F still holds them here); feeds
        # the e_t chain on gpsimd, two classes per wave to bound SBUF
        IND2 = pool.tile([P, 2, KCH, W], FP32)
        ET01 = pool.tile([P, KCH, W], FP32)
        ET23 = pool.tile([P, KCH, W], FP32)
        ETT = pool.tile([P, KCH, W], FP32)
        for c in range(2):
            nc.scalar.activation(IND2[:, c], F[:, c], ACT.Identity,
                                 bias=1.0, scale=-1.0 / BIG)
        nc.gpsimd.tensor_tensor(ET01[:], IND2[:, 0], E[:, 0], op=OP.mult)
        nc.gpsimd.tensor_tensor(ETT[:], IND2[:, 1], E[:, 1], op=OP.mult)
        nc.gpsimd.tensor_tensor(ET01[:], ET01[:], ETT[:], op=OP.add)
        for c in range(2):
            nc.scalar.activation(IND2[:, c], F[:, 2 + c], ACT.Identity,
                                 bias=1.0, scale=-1.0 / BIG)
        nc.gpsimd.tensor_tensor(ET23[:], IND2[:, 0], E[:, 2], op=OP.mult)
        nc.gpsimd.tensor_tensor(ETT[:], IND2[:, 1], E[:, 3], op=OP.mult)
        nc.gpsimd.tensor_tensor(ET23[:], ET23[:], ETT[:], op=OP.add)
        ET = ET01
        nc.gpsimd.tensor_tensor(ET[:], ET01[:], ET23[:], op=OP.add)
        for c in range(C):
            nc.vector.tensor_copy(CUR[:, c], D1T[:, c])
            nc.vector.tensor_tensor(
                CUR[:, c, :, 0:W - 1], CUR[:, c, :, 0:W - 1],
                TMPB[:, c, :, 0:W - 1], op=OP.min)
            nc.vector.tensor_tensor(
                CUR[:, c, :, 2:W], CUR[:, c, :, 2:W],
                TMPB[:, c, :, 0:W - 2], op=OP.min)

        # ---- col pass (deltas 2..K; delta=1 done above per class) ----
        TMPA = FW  # FW (rho) is dead once SQ is built; reuse for even-d adds
        nc.vector.scalar_tensor_tensor(
            CUR[:, :, :, 1:2], D1T[:, :, :, 0:1], 1.0,
            CUR[:, :, :, 1:2], op0=OP.add, op1=OP.min)
        for d in range(2, K + 1):
            bias = BIASQ[:, d - 1:d]
            if d == 2:
                for c in range(C):
                    nc.scalar.activation(TMPA[:, c], D1T[:, c], ACT.Identity,
                                         bias=bias, scale=1.0)
                    nc.vector.tensor_tensor(
                        CUR[:, c, :, 2:], CUR[:, c, :, 2:],
                        TMPA[:, c, :, :W - 2], op=OP.min)
                    nc.vector.tensor_tensor(
                        CUR[:, c, :, :W - 2], CUR[:, c, :, :W - 2],
                        TMPA[:, c, :, 2:], op=OP.min)
                continue
            if d % 2 == 0:
                nc.scalar.activation(TMPA[:], D1T[:], ACT.Identity,
                                     bias=bias, scale=1.0)
                nc.vector.tensor_tensor(
                    CUR[:, :, :, d:], CUR[:, :, :, d:],
                    TMPA[:, :, :, :W - d], op=OP.min)
                nc.vector.tensor_tensor(
                    CUR[:, :, :, :W - d], CUR[:, :, :, :W - d],
                    TMPA[:, :, :, d:], op=OP.min)
            else:
                # TMPB[j] = d1[j+1] + d^2
                nc.scalar.activation(TMPB[:, :, :, 0:W - 1],
                                     D1T[:, :, :, 1:W], ACT.Identity,
                                     bias=bias, scale=1.0)
                # up-shift: x in [0, W-d): candidate d1[x+d] = TMPB[x+d-1]
                nc.vector.tensor_tensor(
                    CUR[:, :, :, 0:W - d], CUR[:, :, :, 0:W - d],
                    TMPB[:, :, :, d - 1:W - 1], op=OP.min)
                # down-shift: x in [d+1, W): candidate d1[x-d] = TMPB[x-d-1]
                nc.vector.tensor_tensor(
                    CUR[:, :, :, d + 1:W], CUR[:, :, :, d + 1:W],
                    TMPB[:, :, :, 0:W - d - 1], op=OP.min)
                # x = d column: candidate d1[0] + d^2 (not in TMPB)
                nc.vector.scalar_tensor_tensor(
                    CUR[:, :, :, d:d + 1], D1T[:, :, :, 0:1], float(d * d),
                    CUR[:, :, :, d:d + 1], op0=OP.add, op1=OP.min)

        # ---- m2' = second-smallest D and max-D certificate, on the bf16
        # transposed maps (2x mode; sqrt commutes with the order stats) ----
        A2 = pool.tile([P, KCH, W], BF16)
        B2 = pool.tile([P, KCH, W], BF16)
        C2 = pool.tile([P, KCH, W], BF16)
        D2 = pool.tile([P, KCH, W], BF16)
        M2T = pool.tile([P, KCH, W], BF16)
        nc.vector.tensor_tensor(A2[:], CUR[:, 0], CUR[:, 1], op=OP.min)
        nc.vector.tensor_tensor(B2[:], CUR[:, 0], CUR[:, 1], op=OP.max)
        nc.vector.tensor_tensor(C2[:], CUR[:, 2], CUR[:, 3], op=OP.min)
        nc.vector.tensor_tensor(D2[:], CUR[:, 2], CUR[:, 3], op=OP.max)
        XM = A2  # after the network, A2 is free to hold the max map
        nc.vector.tensor_tensor(M2T[:], A2[:], C2[:], op=OP.max)
        nc.vector.tensor_tensor(C2[:], B2[:], D2[:], op=OP.min)
        nc.vector.tensor_tensor(B2[:], B2[:], D2[:], op=OP.max)  # max_c D
        nc.vector.tensor_tensor(M2T[:], M2T[:], C2[:], op=OP.min)  # secondmin
        OUT = pool.tile([P, 2], FP32)
        nc.vector.tensor_reduce(OUT[:, 1:2], B2[:], axis=mybir.AxisListType.XY,
                                op=OP.max)

        # ---- transpose back with sqrt on PSUM eviction: R_c + m2 ----
        R = pool.tile([P, C, KCH, W], FP32)
        M2N = pool.tile([P, KCH, W], FP32)
        for c in range(C):
            ps9 = psum_pool_b.tile([P, KCH, KCH, P], BF16, tag="pstb")
            for kw in range(KCH):
                for kh in range(KCH):
                    nc.tensor.matmul(ps9[:, kw, kh, :],
                                     CUR[:, c, kw, kh * P:(kh + 1) * P],
                                     IDENT[:], is_transpose=True)
            nc.scalar.activation(
                R[:, c].rearrange("p kh (kw x) -> p kh kw x", x=P),
                ps9[:].transpose([0, 2, 1, 3]),
                ACT.Sqrt)
        ps9 = psum_pool_b.tile([P, KCH, KCH, P], BF16, tag="pstb")
        for kw in range(KCH):
            for kh in range(KCH):
                nc.tensor.matmul(ps9[:, kw, kh, :],
                                 M2T[:, kw, kh * P:(kh + 1) * P],
                                 IDENT[:], is_transpose=True)
        nc.scalar.activation(
            M2N[:].rearrange("p kh (kw x) -> p kh kw x", x=P),
            ps9[:].transpose([0, 2, 1, 3]),
            ACT.Sqrt)

        # ---- S_e (gpsimd tree) and 1/S_e (DVE) ----
        SE = pool.tile([P, KCH, W], FP32)
        S23 = pool.tile([P, KCH, W], FP32)
        nc.gpsimd.tensor_tensor(SE[:], E[:, 0], E[:, 1], op=OP.add)
        nc.gpsimd.tensor_tensor(S23[:], E[:, 2], E[:, 3], op=OP.add)
        nc.gpsimd.tensor_tensor(SE[:], SE[:], S23[:], op=OP.add)
        RC = pool.tile([P, KCH, W], FP32)
        nc.vector.reciprocal(RC[:], SE[:])

        # ---- e_t * m2 on gpsimd (ET ready early, m2 just arrived) ----
        TPC = pool.tile([P, KCH, W], FP32)
        nc.gpsimd.tensor_tensor(TPC[:], ET[:], M2N[:], op=OP.mult)

        # ---- numerator N = sum_c e_c R_c + S_e - e_t*m2 (DVE tail) ----
        PAC = pool.tile([P, KCH, W], FP32)
        TM = pool.tile([P, KCH, W], FP32)
        nc.vector.tensor_tensor(PAC[:], E[:, 0], R[:, 0], op=OP.mult)
        for c in range(1, C):
            nc.vector.tensor_tensor(TM[:], E[:, c], R[:, c], op=OP.mult)
            nc.vector.tensor_tensor(PAC[:], PAC[:], TM[:], op=OP.add)
        nc.vector.tensor_tensor(PAC[:], PAC[:], SE[:], op=OP.add)
        nc.vector.tensor_tensor(PAC[:], PAC[:], TPC[:], op=OP.subtract)

        # ---- per-partition sums of N / S_e (DVE) ----
        VS = pool.tile([P, KCH, W], FP32)
        nc.vector.scalar_tensor_tensor(VS[:], PAC[:], 1.0, RC[:],
                                       op0=OP.mult, op1=OP.mult,
                                       accum_out=OUT[:, 0:1])
        nc.sync.dma_start(out_d[:], OUT[:])

    nc.finalize()
    return nc


_NC_CACHE: dict[int, bass.Bass] = {}


def _get_nc(K: int) -> bass.Bass:
    if K not in _NC_CACHE:
        _NC_CACHE[K] = _build_nc(K)
    return _NC_CACHE[K]


def _run_device(logits: np.ndarray, targets: np.ndarray, K: int, **kw):
    nc = _get_nc(K)
    in_maps = [
        {"logits": np.ascontiguousarray(logits[b], dtype=np.float32),
         "targets": np.ascontiguousarray(targets[b], dtype=np.int32)}
        for b in range(N_CORES)
    ]
    return run_bass_kernel_spmd(nc, in_maps, list(range(N_CORES)), **kw)


# ---------------------------------------------------------------------------
# exact host fallback (degenerate masks / failed certificate; ~never taken)
# ---------------------------------------------------------------------------

def _edt2_exact_np(mask: np.ndarray) -> np.ndarray:
    """Exact squared EDT to nearest True pixel (brute-force separable,
    float64; matches the reference's construction)."""
    Hh, Ww = mask.shape
    f = np.where(mask, 0.0, 1e8)
    iw = np.arange(Ww, dtype=np.float64)
    sqw = (iw[:, None] - iw[None, :]) ** 2
    d1 = (f[:, None, :] + sqw[None, :, :]).min(axis=-1)
    ih = np.arange(Hh, dtype=np.float64)
    sqh = (ih[:, None] - ih[None, :]) ** 2
    d2 = (d1[None, :, :] + sqh[:, :, None]).min(axis=1)
    return d2


def _loss_host_exact(logits: np.ndarray, targets: np.ndarray) -> np.float32:
    B = logits.shape[0]
    lo = logits.astype(np.float64)
    mx = lo.max(axis=1, keepdims=True)
    e = np.exp(lo - mx)
    probs = e / e.sum(axis=1, keepdims=True)
    total = 0.0
    for b in range(B):
        for c in range(C):
            m = targets[b] == c
            s = int(m.sum())
            pos = np.sqrt(_edt2_exact_np(m))
            if s == 0:
                phi = pos
            elif s == m.size:
                phi = -np.sqrt(_edt2_exact_np(~m))
            else:
                phi = pos - np.sqrt(_edt2_exact_np(~m)) + 1.0
            total += float((probs[b, c] * phi).sum())
    return np.float32(total / (B * C * H * W))


def kernel(logits: np.ndarray, targets: np.ndarray) -> np.ndarray:
    logits = np.asarray(logits)
    targets = np.asarray(targets)
    assert logits.shape == (N_CORES, C, H, W) and targets.shape == (N_CORES, H, W)

    # degenerate masks (empty/full class) take the reference's special
    # branches -- handle on host (measure-zero for the target distribution)
    counts = np.stack([(targets == c).sum(axis=(1, 2)) for c in range(C)])
    if counts.min() == 0 or counts.max() == H * W:
        return np.asarray(_loss_host_exact(logits, targets))

    K = DEFAULT_K
    for _attempt in range(3):
        res = _run_device(logits, targets, K).results
        out = np.stack([res[b]["out"] for b in range(N_CORES)])  # (8, 128, 2)
        maxd = float(out[:, :, 1].max())
        if maxd <= K * K:
            total = float(out[:, :, 0].astype(np.float64).sum())
            return np.asarray(np.float32(total / (N_CORES * C * H * W)))
        if maxd > 4000.0 * 4000.0:  # sentinel leaked: window saw no features
            break
        K = int(np.ceil(np.sqrt(maxd))) + 1
    return np.asarray(_loss_host_exact(logits, targets))



# revision 9
# speedup vs baseline: 1.0564x; 1.0564x over previous
"""Boundary-loss Trainium2 kernel (Bass/Tile), SPMD over 8 NeuronCores.

loss = mean(softmax(logits, C) * phi(targets)), phi the signed EDT map.
Per pixel p with target t:  sum_c probs_c*phi_c = (sum_c e_c R_c - e_t*m2)/S_e + 1
with R_c = sqrt(edt2(mask_c)), m2 = min_{c!=t} R_c (= second-smallest R).

Device algorithm (one batch item per core, bf16 maps unless noted):
  * masks F_c = (t != c)*BIG with BIG pad columns; the 1-D L1 row distance
    runs as TWO flattened tensor_tensor_scans (fw + reversed bw) on DVE,
    then one strided row-min.
  * PE transposes logits (fp32) and rho (bf16) blockwise; ACT evicts PSUM
    fused with Exp (E = e^logit) resp. Square (d1 = rho^2). Everything
    downstream stays in transposed space -- no back-transposes.
  * col pass: exact windowed parabolic mins, window K: ACT/DVE prebuild
    TMPA_d = d1 + d^2, DVE runs one merged in-place 2x-mode min chain over
    all 4 classes (2 shifted mins per delta).
  * R = sqrt(D) on ACT; order stats run on R, so m2 needs no extra sqrt and
    the exactness certificate is max(R) <= K+1 (any pixel whose computed D
    is <= (K+1)^2 is provably exact; host retries with K+1 else).
  * tail on raw e_c (softmax never materialized): e_t via [d1==0]
    indicators (DVE 4x tensor_scalar) with mult/add chains on GPSIMD;
    S = sum_c e_c R_c; two fused scalar_tensor_tensor accumulations fold
    the single 1/S_e map into per-partition sums of S/S_e and e_t*m2/S_e;
    the host subtracts them and adds the +1/C term.
"""
from contextlib import ExitStack

import numpy as np

import concourse.bass as bass
import concourse.tile as tile
from concourse import bacc, mybir
from concourse.bass_utils import run_bass_kernel_spmd
from concourse.masks import make_identity

P = 128
C = 4
H = W = 384
KCH = H // P     # 3 row chunks (natural space)
KW = W // P      # 3 col chunks (transposed space)
PAD = 8
WP = W + PAD     # padded row length for the flattened scans
FLAT = C * KCH * WP
N_CORES = 8
BIG = 65536.0
DEFAULT_K = 4    # parabolic window; exact iff max R <= K+1 (certified)

FP32 = mybir.dt.float32
BF16 = mybir.dt.bfloat16
INT32 = mybir.dt.int32
OP = mybir.AluOpType
ACT = mybir.ActivationFunctionType


def _build_nc(K: int) -> bass.Bass:
    nc = bacc.Bacc("TRN2", target_bir_lowering=False, debug=False)
    logits_d = nc.dram_tensor("logits", [C, H, W], FP32, kind="ExternalInput")
    targets_d = nc.dram_tensor("targets", [H, W], INT32, kind="ExternalInput")
    out_d = nc.dram_tensor("out", [P, 4], FP32, kind="ExternalOutput")

    with tile.TileContext(nc) as tc, ExitStack() as ctx:
        pool = ctx.enter_context(tc.tile_pool(name="main", bufs=1))
        psq = ctx.enter_context(tc.tile_pool(name="psq", bufs=2, space="PSUM"))
        psl = ctx.enter_context(tc.tile_pool(name="psl", bufs=1, space="PSUM"))

        # ---- loads ----
        T = pool.tile([P, KCH, W], INT32)
        tr = targets_d[:].rearrange("(k p) w -> p k w", p=P)
        for k in range(KCH):
            nc.sync.dma_start(T[:, k], tr[:, k])
        L = pool.tile([P, C, KCH, W], FP32)
        nc.sync.dma_start(L[:], logits_d[:].rearrange("c (k p) w -> p c k w", p=P))

        # ---- constants ----
        ONES = pool.tile([P, FLAT], BF16)
        nc.vector.memset(ONES[:], 1.0)
        IDENT = pool.tile([P, P], BF16)
        make_identity(nc, IDENT[:])
        IDF = pool.tile([P, P], FP32)
        make_identity(nc, IDF[:])
        BIASQ = pool.tile([P, K], FP32)
        for d in range(2, K + 1):
            nc.vector.memset(BIASQ[:, d - 1:d], float(d * d))

        # ---- masks F_c = (t != c)*BIG, with BIG pad columns ----
        F = pool.tile([P, C, KCH, WP], BF16)
        nc.gpsimd.memset(F[:, :, :, W:WP], BIG)
        TFb = pool.tile([P, KCH, W], BF16)
        for k in range(KCH):
            nc.scalar.copy(TFb[:, k], T[:, k])
        for c in range(C):
            nc.vector.tensor_scalar(F[:, c, :, 0:W], TFb[:], float(c), BIG,
                                    op0=OP.not_equal, op1=OP.mult)

        # ---- row pass: flattened L1 scans, then min ----
        FW = pool.tile([P, C, KCH, WP], BF16)
        BW = pool.tile([P, C, KCH, WP], BF16)
        ff = F[:].rearrange("p c k w -> p (c k w)")
        fwf = FW[:].rearrange("p c k w -> p (c k w)")
        bwf = BW[:].rearrange("p c k w -> p (c k w)")
        nc.vector.tensor_tensor_scan(fwf, ONES[:], ff, BIG,
                                     op0=OP.add, op1=OP.min)
        nc.vector.tensor_tensor_scan(bwf[:, ::-1], ONES[:], ff[:, ::-1], BIG,
                                     op0=OP.add, op1=OP.min)
        nc.vector.tensor_tensor(FW[:, :, :, 0:W], FW[:, :, :, 0:W],
                                BW[:, :, :, 0:W], op=OP.min)  # rho

        # ---- PE transposes + fused evictions ----
        # logits first (PE is idle while scans run; E feeds the mid chains),
        # then rho per class (feeds the col pass).
        ET4 = pool.tile([P, C, KW, H], BF16)   # e^logit, transposed
        D1T = pool.tile([P, C, KW, H], BF16)   # rho^2, transposed
        for c in range(C):
            pl = psl.tile([P, KW, KCH, P], FP32, tag="psl")
            for kw in range(KW):
                for kh in range(KCH):
                    nc.tensor.matmul(pl[:, kw, kh, :],
                                     L[:, c, kh, kw * P:(kw + 1) * P],
                                     IDF[:], is_transpose=True)
            nc.scalar.activation(
                ET4[:, c], pl[:].rearrange("p kw kh x -> p kw (kh x)"),
                ACT.Exp)
        for c in range(C):
            p9 = psq.tile([P, KW, KCH, P], BF16, tag="ps9")
            for kw in range(KW):
                for kh in range(KCH):
                    nc.tensor.matmul(p9[:, kw, kh, :],
                                     FW[:, c, kh, kw * P:(kw + 1) * P],
                                     IDENT[:], is_transpose=True)
            nc.scalar.activation(
                D1T[:, c], p9[:].rearrange("p kw kh x -> p kw (kh x)"),
                ACT.Square)

        # ---- S_e and 1/S_e (fills the DVE gap while rho transposes run;
        # 1/S_e folds into the two final fused accumulations) ----
        SE = pool.tile([P, KW, H], BF16)
        TMP = pool.tile([P, KW, H], BF16)
        nc.vector.tensor_tensor(SE[:], ET4[:, 0], ET4[:, 1], op=OP.add)
        nc.vector.tensor_tensor(TMP[:], ET4[:, 2], ET4[:, 3], op=OP.add)
        nc.vector.tensor_tensor(SE[:], SE[:], TMP[:], op=OP.add)
        RC = pool.tile([P, KW, H], FP32)
        nc.vector.reciprocal(RC[:], SE[:])

        # ---- e_t chain: [d1==0] indicators (DVE 4x), mult/add on Pool ----
        IND = pool.tile([P, C, KW, H], BF16)
        nc.vector.tensor_scalar(IND[:], D1T[:], 0.0, None, op0=OP.is_equal)
        IE = pool.tile([P, C, KW, H], BF16)
        for c in range(C):
            nc.gpsimd.tensor_tensor(IE[:, c], IND[:, c], ET4[:, c], op=OP.mult)
        ETP = pool.tile([P, KW, H], BF16)      # e_t (raw)
        ET2 = pool.tile([P, KW, H], BF16)
        nc.gpsimd.tensor_tensor(ETP[:], IE[:, 0], IE[:, 1], op=OP.add)
        nc.gpsimd.tensor_tensor(ET2[:], IE[:, 2], IE[:, 3], op=OP.add)
        nc.gpsimd.tensor_tensor(ETP[:], ETP[:], ET2[:], op=OP.add)

        # ---- col pass: TMPA_d = d1 + d^2 (d=1 on DVE at 4x, rest on ACT),
        # one merged in-place 2x min chain over all 4 classes ----
        TMPA = {}
        for d in range(1, K + 1):
            tmpa_d = pool.tile([P, C, KW, H], BF16, name=f"tmpa{d}")
            TMPA[d] = tmpa_d
        nc.vector.tensor_scalar(TMPA[1][:], D1T[:], 1.0, None, op0=OP.add)
        for d in range(2, K + 1):
            nc.scalar.activation(TMPA[d][:], D1T[:], ACT.Identity,
                                 bias=BIASQ[:, d - 1:d], scale=1.0)

        CUR = pool.tile([P, C, KW, H], BF16)
        R = pool.tile([P, C, KW, H], BF16)
        nc.vector.tensor_scalar(CUR[:, :, :, H - 1:H],
                                D1T[:, :, :, H - 1:H], 0.0, None, op0=OP.add)
        nc.vector.tensor_tensor(CUR[:, :, :, 0:H - 1], D1T[:, :, :, 0:H - 1],
                                TMPA[1][:, :, :, 1:H], op=OP.min)
        nc.vector.tensor_tensor(CUR[:, :, :, 1:H], CUR[:, :, :, 1:H],
                                TMPA[1][:, :, :, 0:H - 1], op=OP.min)
        for d in range(2, K + 1):
            nc.vector.tensor_tensor(
                CUR[:, :, :, 0:H - d], CUR[:, :, :, 0:H - d],
                TMPA[d][:, :, :, d:H], op=OP.min)
            nc.vector.tensor_tensor(
                CUR[:, :, :, d:H], CUR[:, :, :, d:H],
                TMPA[d][:, :, :, 0:H - d], op=OP.min)
        nc.scalar.activation(R[:], CUR[:], ACT.Sqrt)

        # ---- order stats on R: m2 = second-smallest, cert = max ----
        A2 = pool.tile([P, KW, H], BF16)
        B2 = pool.tile([P, KW, H], BF16)
        C2 = pool.tile([P, KW, H], BF16)
        D2 = pool.tile([P, KW, H], BF16)
        M2 = pool.tile([P, KW, H], BF16)
        OUT = pool.tile([P, 4], FP32)
        nc.vector.tensor_tensor(A2[:], R[:, 0], R[:, 1], op=OP.min)
        nc.vector.tensor_tensor(B2[:], R[:, 0], R[:, 1], op=OP.max)
        nc.vector.tensor_tensor(C2[:], R[:, 2], R[:, 3], op=OP.min)
        nc.vector.tensor_tensor(D2[:], R[:, 2], R[:, 3], op=OP.max)
        nc.vector.tensor_tensor(M2[:], A2[:], C2[:], op=OP.max)
        nc.vector.tensor_tensor(C2[:], B2[:], D2[:], op=OP.min)
        nc.vector.tensor_tensor(B2[:], B2[:], D2[:], op=OP.max)  # max R
        nc.vector.tensor_tensor(M2[:], M2[:], C2[:], op=OP.min)  # secondmin
        nc.vector.tensor_reduce(OUT[:, 1:2], B2[:], axis=mybir.AxisListType.XY,
                                op=OP.max)

        # ---- S = sum_c e_c R_c ; accumulate S/S_e and e_t*m2/S_e ----
        G0 = pool.tile([P, KW, H], BF16)
        G1 = pool.tile([P, KW, H], BF16)
        G2 = pool.tile([P, KW, H], BF16)
        G3 = pool.tile([P, KW, H], BF16)
        XM = pool.tile([P, KW, H], BF16)
        nc.gpsimd.tensor_tensor(G2[:], ET4[:, 2], R[:, 2], op=OP.mult)
        nc.gpsimd.tensor_tensor(G3[:], ET4[:, 3], R[:, 3], op=OP.mult)
        nc.gpsimd.tensor_tensor(G2[:], G2[:], G3[:], op=OP.add)
        nc.gpsimd.tensor_tensor(XM[:], ETP[:], M2[:], op=OP.mult)
        nc.vector.tensor_tensor(G0[:], ET4[:, 0], R[:, 0], op=OP.mult)
        nc.vector.tensor_tensor(G1[:], ET4[:, 1], R[:, 1], op=OP.mult)
        nc.vector.tensor_tensor(G0[:], G0[:], G1[:], op=OP.add)
        nc.vector.tensor_tensor(G0[:], G0[:], G2[:], op=OP.add)
        JUNK = G1
        nc.vector.scalar_tensor_tensor(JUNK[:], G0[:], 1.0, RC[:],
                                       op0=OP.mult, op1=OP.mult,
                                       accum_out=OUT[:, 0:1])
        nc.vector.scalar_tensor_tensor(JUNK[:], XM[:], 1.0, RC[:],
                                       op0=OP.mult, op1=OP.mult,
                                       accum_out=OUT[:, 2:3])
        nc.vector.memset(OUT[:, 3:4], 0.0)
        nc.sync.dma_start(out_d[:], OUT[:])

    nc.finalize()
    return nc


_NC_CACHE: dict[int, bass.Bass] = {}


def _get_nc(K: int) -> bass.Bass:
    if K not in _NC_CACHE:
        _NC_CACHE[K] = _build_nc(K)
    return _NC_CACHE[K]


def _run_device(logits: np.ndarray, targets: np.ndarray, K: int, **kw):
    nc = _get_nc(K)
    in_maps = [
        {"logits": np.ascontiguousarray(logits[b], dtype=np.float32),
         "targets": np.ascontiguousarray(targets[b], dtype=np.int32)}
        for b in range(N_CORES)
    ]
    return run_bass_kernel_spmd(nc, in_maps, list(range(N_CORES)), **kw)


# ---------------------------------------------------------------------------
# exact host fallback (degenerate masks / failed certificate; ~never taken)
# ---------------------------------------------------------------------------

def _edt2_exact_np(mask: np.ndarray) -> np.ndarray:
    Hh, Ww = mask.shape
    f = np.where(mask, 0.0, 1e8)
    iw = np.arange(Ww, dtype=np.float64)
    sqw = (iw[:, None] - iw[None, :]) ** 2
    d1 = (f[:, None, :] + sqw[None, :, :]).min(axis=-1)
    ih = np.arange(Hh, dtype=np.float64)
    sqh = (ih[:, None] - ih[None, :]) ** 2
    d2 = (d1[None, :, :] + sqh[:, :, None]).min(axis=1)
    return d2


def _loss_host_exact(logits: np.ndarray, targets: np.ndarray) -> np.float32:
    B = logits.shape[0]
    lo = logits.astype(np.float64)
    mx = lo.max(axis=1, keepdims=True)
    e = np.exp(lo - mx)
    probs = e / e.sum(axis=1, keepdims=True)
    total = 0.0
    for b in range(B):
        for c in range(C):
            m = targets[b] == c
            s = int(m.sum())
            pos = np.sqrt(_edt2_exact_np(m))
            if s == 0:
                phi = pos
            elif s == m.size:
                phi = -np.sqrt(_edt2_exact_np(~m))
            else:
                phi = pos - np.sqrt(_edt2_exact_np(~m)) + 1.0
            total += float((probs[b, c] * phi).sum())
    return np.float32(total / (B * C * H * W))


def kernel(logits: np.ndarray, targets: np.ndarray) -> np.ndarray:
    logits = np.asarray(logits)
    targets = np.asarray(targets)
    assert logits.shape == (N_CORES, C, H, W) and targets.shape == (N_CORES, H, W)

    counts = np.stack([(targets == c).sum(axis=(1, 2)) for c in range(C)])
    if counts.min() == 0 or counts.max() == H * W:
        return np.asarray(_loss_host_exact(logits, targets))

    K = DEFAULT_K
    for _attempt in range(3):
        res = _run_device(logits, targets, K).results
        out = np.stack([res[b]["out"] for b in range(N_CORES)])  # (8, 128, 4)
        maxr = float(out[:, :, 1].max())
        # cert: every computed D with sqrt <= K+1 is provably exact
        if maxr <= (K + 1) + 1e-3:
            total = (float(out[:, :, 0].astype(np.float64).sum())
                     - float(out[:, :, 2].astype(np.float64).sum()))
            return np.asarray(
                np.float32(total / (N_CORES * C * H * W) + 1.0 / C))
        if maxr > 4000.0:  # sentinel leaked: a window saw no features
            break
        K = int(np.ceil(maxr))
    return np.asarray(_loss_host_exact(logits, targets))
